# revision 22
# baseline (speedup 1.0000x reference)
"""Trainium2 Bass kernel for nn_LinearSelfAttention (linear attention w/ RoPE,
elu+1 feature map, qkv + out projections).

Sharding: 8 cores = 4 batches x 2 head-groups (8 heads each).
Each core computes, for its (batch b, head-group g):
  qkv slice projection, RoPE, feature maps, per-head kv state (64x64),
  attention output, and a partial out-projection (its heads' rows of W_out).
Host sums the two head-group partials per batch and transposes.

Layout strategy (everything feature-major where matmuls need it):
  - host passes xT = x[b].T (D=1024, T=4096), fp16
  - q is produced in (d, t) layout (W_q stationary, xT moving)
  - k, v are produced in (t, d) layout (xT tiles stationary, W_kv moving)
  - head-dims are interleaved (d, d+32) -> (2j, 2j+1) by permuting W_q/W_k
    columns on the host so the rotate-half partner is the XOR-1 partition
    (reachable by DVE stream_shuffle) / XOR-1 free element.
  - all elementwise stage-B work runs in fp16 (2x DVE mode); PSUM fp32 is
    cast down by one copy per tile, which also releases the PSUM bank early.
  - denominators: a (128,128) block stationary replicates k_sum across 64
    output rows, so the dps matmul emits the reciprocal input already
    broadcast to the (dims, tokens) layout -- no select/broadcast step.
"""

import sys

sys.path.insert(0, "/opt/trn_rl_repo")

import ml_dtypes
import numpy as np

import concourse.bacc as bacc
import concourse.mybir as mybir
from concourse.tile import TileContext
from concourse.bass_utils import run_bass_kernel_spmd

FP32 = mybir.dt.float32
F16 = mybir.dt.float16
AF = mybir.ActivationFunctionType
ALU = mybir.AluOpType

DIM = 1024
T = 4096
HEADS = 16
HD = 64
H_CORE = 8  # heads per core
ROPE_BASE = 500000.0
SCALE = HD**-0.5

NCH = 8  # chunks over T
CH = T // NCH  # 512 tokens per chunk
NSUB = CH // 128  # 4 sub-chunks of 128 tokens
DC = DIM // 128  # 8 contraction tiles
NPAIR = H_CORE // 2  # 4 head pairs (128 dims each)

_SHUF_MASK = [i ^ 1 for i in range(32)]


def _build():
    nc = bacc.Bacc(None, target_bir_lowering=False, debug=False)

    xT = nc.declare_dram_parameter("xT", [DIM, T], F16, isOutput=False)
    wq = nc.declare_dram_parameter("wq", [DIM, 512], F16, isOutput=False)
    wkv = nc.declare_dram_parameter("wkv", [DIM, 1024], F16, isOutput=False)
    wo = nc.declare_dram_parameter("wo", [512, DIM], F16, isOutput=False)
    cosq = nc.declare_dram_parameter("cosq", [128, T], F16, isOutput=False)
    sinq = nc.declare_dram_parameter("sinq", [128, T], F16, isOutput=False)
    cosk = nc.declare_dram_parameter("cosk", [128, 32 * HD], F16, isOutput=False)
    sink = nc.declare_dram_parameter("sink", [128, 32 * HD], F16, isOutput=False)
    ones16 = nc.declare_dram_parameter("ones16", [128, 4], F16, isOutput=False)
    outT = nc.declare_dram_parameter("outT", [DIM, T], F16, isOutput=True)

    with TileContext(nc) as tc, nc.allow_low_precision(
        reason="fp16 tiles feed matmuls; rounding is intended"
    ):
        with tc.tile_pool(name="persist", bufs=1) as persist:
            # all 4 pairs' q features side by side: pair p at cols [p*T, (p+1)*T)
            qf = persist.tile([128, NPAIR * T], F16, tag="qf", name="qf")
            wo_t = [
                persist.tile([128, 1024], F16, tag=f"wo{p}", name=f"wo{p}")
                for p in range(NPAIR)
            ]
            bdiag = [
                persist.tile([128, 128], F16, tag=f"bd{p}", name=f"bd{p}")
                for p in range(NPAIR)
            ]
            den_f = [
                persist.tile([128, 128], F16, tag=f"dn{p}", name=f"dn{p}")
                for p in range(NPAIR)
            ]
            for p in range(NPAIR):
                nc.gpsimd.memset(bdiag[p][:], 0.0)
                nc.gpsimd.memset(den_f[p][:], 0.0)

            with tc.tile_pool(name="pskv", bufs=1, space="PSUM") as pskv:
                kvps = [
                    pskv.tile([128, 258], FP32, tag=f"kv{p}", name=f"kv{p}")
                    for p in range(NPAIR)
                ]

                # ---------------- phase 1 ----------------
                with tc.tile_pool(name="w1", bufs=1) as w1, tc.tile_pool(
                    name="s1", bufs=2
                ) as s1, tc.tile_pool(name="ps1", bufs=1, space="PSUM") as ps1:

                    def load_chunk(c, xt=None, teng=None):
                        teng = teng or nc.gpsimd
                        tsl = slice(c * CH, (c + 1) * CH)
                        if xt is None:
                            xt = []
                            for d in range(DC):
                                t_ = s1.tile([128, CH], F16, tag=f"x{d}", name=f"x{d}")
                                nc.sync.dma_start(
                                    out=t_[:], in_=xT[d * 128 : (d + 1) * 128, tsl]
                                )
                                xt.append(t_)
                        cq = s1.tile([128, CH], F16, tag="cq", name="cq")
                        sq = s1.tile([128, CH], F16, tag="sq", name="sq")
                        teng.dma_start(out=cq[:], in_=cosq[:, tsl])
                        teng.dma_start(out=sq[:], in_=sinq[:, tsl])
                        ksl = slice(c * NSUB * HD, (c + 1) * NSUB * HD)
                        ck = s1.tile([128, NSUB * HD], F16, tag="ck", name="ck")
                        sk = s1.tile([128, NSUB * HD], F16, tag="sk", name="sk")
                        teng.dma_start(out=ck[:], in_=cosk[:, ksl])
                        teng.dma_start(out=sk[:], in_=sink[:, ksl])
                        return xt, cq, sq, ck, sk

                    # startup: wq + x chunk0 interleaved on the sync queue
                    # (first matmul only needs wq0+x0); wkv on the scalar
                    # queue so the first kv matmul is never starved.
                    # startup: balance ~4.4MB of initial loads across all 3
                    # DMA queues (~116GB/s each), in consumption order.
                    wq_t = []
                    xt0 = []
                    for d in range(DC):
                        w_ = w1.tile([128, 512], F16, tag=f"wq{d}", name=f"wq{d}")
                        x_ = s1.tile([128, CH], F16, tag=f"x{d}", name=f"x{d}")
                        eng = nc.sync if d < 4 else nc.gpsimd
                        eng.dma_start(out=w_[:], in_=wq[d * 128 : (d + 1) * 128, :])
                        eng.dma_start(out=x_[:], in_=xT[d * 128 : (d + 1) * 128, 0:CH])
                        wq_t.append(w_)
                        xt0.append(x_)
                    wkv_t = []
                    for d in range(DC):
                        t_ = w1.tile([128, 1024], F16, tag=f"wkv{d}", name=f"wkv{d}")
                        eng = (nc.scalar, nc.scalar, nc.scalar, nc.scalar,
                               nc.sync, nc.sync, nc.gpsimd, nc.gpsimd)[d]
                        eng.dma_start(out=t_[:], in_=wkv[d * 128 : (d + 1) * 128, :])
                        wkv_t.append(t_)
                    chunk0 = load_chunk(0, xt0, teng=nc.scalar)

                    pend_c = []  # deferred stage-C batches: (kfall, vxs, c)

                    def emit_stage_c(n=1):
                        for _ in range(min(n, len(pend_c))):
                            kfall_, vxs_, c_ = pend_c.pop(0)
                            for s4 in range(NSUB):
                                s_ = c_ * NSUB + s4
                                vx_ = vxs_[s4]
                                for p in range(NPAIR):
                                    rhs = (
                                        vx_[:, 0:258] if p < 2 else vx_[:, 258:516]
                                    )
                                    nc.tensor.matmul(
                                        kvps[p][:],
                                        kfall_[
                                            :,
                                            s4 * 512 + p * 128 : s4 * 512 + (p + 1) * 128,
                                        ],
                                        rhs,
                                        start=(s_ == 0),
                                        stop=(s_ == NCH * NSUB - 1),
                                        skip_group_check=True,
                                    )

                    def emit_q_mm(ct, xt, q16all):
                        pq = ps1.tile([128, CH], FP32, tag="pq", bufs=2)
                        for d in range(DC):
                            nc.tensor.matmul(
                                pq[:],
                                wq_t[d][:, ct * 128 : (ct + 1) * 128],
                                xt[d][:],
                                start=(d == 0),
                                stop=(d == DC - 1),
                            )
                        # single PSUM reader: releases the bank fast
                        nc.vector.tensor_copy(
                            out=q16all[:, ct * CH : (ct + 1) * CH], in_=pq[:]
                        )

                    def emit_q_tails(q16all, cq, sq, c):
                        tsl = slice(c * CH, (c + 1) * CH)

                        def g3(ap):
                            return ap.rearrange("p (g t) -> p g t", g=NPAIR)

                        cqb = cq[:].unsqueeze(1).broadcast_to([128, NPAIR, CH])
                        sqb = sq[:].unsqueeze(1).broadcast_to([128, NPAIR, CH])
                        sh = s1.tile([128, NPAIR * CH], F16, tag="qA", name="qsh")
                        nc.vector.stream_shuffle(sh[:], q16all[:], _SHUF_MASK)
                        tmp = s1.tile([128, NPAIR * CH], F16, tag="qB", name="qtmp")
                        nc.vector.tensor_mul(g3(tmp[:]), g3(q16all[:]), cqb)
                        u = s1.tile([128, NPAIR * CH], F16, tag="qA", name="qu")
                        nc.vector.tensor_mul(g3(u[:]), g3(sh[:]), sqb)
                        ro = s1.tile([128, NPAIR * CH], F16, tag="qR", name="qro")
                        nc.vector.tensor_add(ro[:], tmp[:], u[:])
                        m = s1.tile([128, NPAIR * CH], F16, tag="qM", name="qm")
                        nc.vector.tensor_scalar_min(m[:], ro[:], 0.0)

                        def finish():
                            e = s1.tile(
                                [128, NPAIR * CH], F16, tag="qB", name="qe"
                            )
                            nc.scalar.activation(e[:], m[:], AF.Exp, scale=1.0)
                            nc.vector.scalar_tensor_tensor(
                                qf[:].rearrange("p (g t) -> p g t", g=NPAIR)[
                                    :, :, tsl
                                ],
                                in0=g3(ro[:]),
                                scalar=0.0,
                                in1=g3(e[:]),
                                op0=ALU.max,
                                op1=ALU.add,
                            )

                        return finish

                    def emit_kv_mm(s4, xt, k16all, vxs):
                        pk = ps1.tile([128, 512], FP32, tag="pk")
                        pv = ps1.tile([128, 512], FP32, tag="pv")
                        for d in range(DC):
                            lhs = xt[d][:, s4 * 128 : (s4 + 1) * 128]
                            nc.tensor.matmul(
                                pk[:],
                                lhs,
                                wkv_t[d][:, 0:512],
                                start=(d == 0),
                                stop=(d == DC - 1),
                                skip_group_check=True,
                            )
                            nc.tensor.matmul(
                                pv[:],
                                lhs,
                                wkv_t[d][:, 512:1024],
                                start=(d == 0),
                                stop=(d == DC - 1),
                                skip_group_check=True,
                            )
                        nc.scalar.copy(
                            out=k16all[:, s4 * 512 : (s4 + 1) * 512], in_=pk[:]
                        )
                        vx = s1.tile([128, 516], F16, tag="vx", name="vx", bufs=12)
                        nc.scalar.copy(
                            out=vx[:].rearrange("p (g c) -> p g c", g=2, c=258)[
                                :, :, 0:256
                            ],
                            in_=pv[:].rearrange("p (g c) -> p g c", g=2, c=256),
                        )
                        nc.gpsimd.dma_start(
                            out=vx[:].rearrange("p (g c) -> p g c", g=2, c=258)[
                                :, :, 256:258
                            ],
                            in_=ones16[:].rearrange("p (g c) -> p g c", g=2),
                        )
                        vxs.append(vx)

                    def emit_k_tails(k16all, ck, sk, kfall):
                        def v4(ap):
                            return ap.rearrange(
                                "p (s h d) -> p s h d", s=NSUB, h=H_CORE
                            )

                        ckb = (
                            ck[:]
                            .rearrange("p (s d) -> p s d", s=NSUB)
                            .unsqueeze(2)
                            .broadcast_to([128, NSUB, H_CORE, HD])
                        )
                        tmpk = s1.tile([128, NSUB * 512], F16, tag="kB", name="ktmp")
                        nc.vector.tensor_mul(v4(tmpk[:]), v4(k16all[:]), ckb)
                        uk = s1.tile([128, NSUB * 512], F16, tag="kC", name="kuk")
                        uk5 = uk[:].rearrange(
                            "p (s h j two) -> p s h j two", s=NSUB, h=H_CORE, j=32
                        )
                        k5 = k16all[:].rearrange(
                            "p (s h j two) -> p s h j two", s=NSUB, h=H_CORE, j=32
                        )
                        sk5 = sk[:].rearrange("p (s j two) -> p s j two", s=NSUB, j=32)
                        for ev in range(2):
                            nc.vector.tensor_mul(
                                uk5[:, :, :, :, ev],
                                k5[:, :, :, :, 1 - ev],
                                sk5[:, :, :, ev : ev + 1]
                                .unsqueeze(2)
                                .broadcast_to([128, NSUB, H_CORE, 32, 1])[
                                    :, :, :, :, 0
                                ],
                            )
                        rok = s1.tile([128, NSUB * 512], F16, tag="kB", name="krok")
                        nc.vector.tensor_add(rok[:], tmpk[:], uk[:])
                        mk = s1.tile([128, NSUB * 512], F16, tag="kC", name="kmk")
                        nc.vector.tensor_scalar_min(mk[:], rok[:], 0.0)
                        ek = s1.tile([128, NSUB * 512], F16, tag="kB", name="kek")
                        nc.scalar.activation(ek[:], mk[:], AF.Exp, scale=1.0)
                        nc.vector.scalar_tensor_tensor(
                            kfall[:],
                            in0=rok[:],
                            scalar=0.0,
                            in1=ek[:],
                            op0=ALU.max,
                            op1=ALU.add,
                        )

                    def emit_k_tail_sub(s4, k16all, ck, sk, kfall):
                        """Narrow (512-wide) k tail for one sub-chunk: short
                        critical chain for the last chunk's kv state."""
                        sl = slice(s4 * 512, (s4 + 1) * 512)
                        ksl = slice(s4 * HD, (s4 + 1) * HD)
                        tmpk = s1.tile([128, 512], F16, tag="kBn", name="ktmpn")
                        nc.vector.tensor_mul(
                            tmpk[:].rearrange("p (h d) -> p h d", h=H_CORE),
                            k16all[:, sl].rearrange("p (h d) -> p h d", h=H_CORE),
                            ck[:, ksl].unsqueeze(1).broadcast_to(
                                [128, H_CORE, HD]
                            ),
                        )
                        uk = s1.tile([128, 512], F16, tag="kCn", name="kukn")
                        uk4 = uk[:].rearrange(
                            "p (h j two) -> p h j two", h=H_CORE, j=32
                        )
                        k4 = k16all[:, sl].rearrange(
                            "p (h j two) -> p h j two", h=H_CORE, j=32
                        )
                        sk4 = sk[:, ksl].rearrange("p (j two) -> p j two", j=32)
                        for ev in range(2):
                            nc.vector.tensor_mul(
                                uk4[:, :, :, ev],
                                k4[:, :, :, 1 - ev],
                                sk4[:, :, ev : ev + 1]
                                .unsqueeze(1)
                                .broadcast_to([128, H_CORE, 32, 1])[:, :, :, 0],
                            )
                        rok = s1.tile([128, 512], F16, tag="kBn", name="krokn")
                        nc.vector.tensor_add(rok[:], tmpk[:], uk[:])
                        mk = s1.tile([128, 512], F16, tag="kCn", name="kmkn")
                        nc.vector.tensor_scalar_min(mk[:], rok[:], 0.0)
                        ek = s1.tile([128, 512], F16, tag="kBn", name="kekn")
                        nc.scalar.activation(ek[:], mk[:], AF.Exp, scale=1.0)
                        nc.vector.scalar_tensor_tensor(
                            kfall[:, sl],
                            in0=rok[:],
                            scalar=0.0,
                            in1=ek[:],
                            op0=ALU.max,
                            op1=ALU.add,
                        )

                    nxt = None  # prefetched loads for the last chunk
                    for c in range(NCH):
                        if c == 0:
                            xt, cq, sq, ck, sk = chunk0
                        elif c == NCH - 1:
                            xt, cq, sq, ck, sk = nxt
                        else:
                            xt, cq, sq, ck, sk = load_chunk(c)
                        if c >= 2:
                            emit_stage_c(1)
                        q16all = s1.tile(
                            [128, NPAIR * CH], F16, tag="q16", name="q16"
                        )
                        k16all = s1.tile(
                            [128, NSUB * 512], F16, tag="k16", name="k16"
                        )
                        kfall = s1.tile(
                            [128, NSUB * 512], F16, tag="kf", name="kf", bufs=3
                        )
                        vxs = []
                        if c < NCH - 1:
                            for ct in range(NPAIR):
                                emit_q_mm(ct, xt, q16all)
                            emit_kv_mm(0, xt, k16all, vxs)
                            fin7 = None
                            if c == NCH - 2:
                                # run the LAST chunk's whole q path now (mid
                                # chunk 6) so its Vector work drains before
                                # the phase transition; its PSUM-freeing
                                # copies must precede the wide tail chains.
                                nxt = load_chunk(NCH - 1)
                                q16b = s1.tile(
                                    [128, NPAIR * CH], F16, tag="q16", name="q16b"
                                )
                                for ct in range(NPAIR):
                                    emit_q_mm(ct, nxt[0], q16b)
                                fin6 = emit_q_tails(q16all, cq, sq, c)
                                fin7 = emit_q_tails(q16b, nxt[1], nxt[2], NCH - 1)
                            else:
                                fin6 = emit_q_tails(q16all, cq, sq, c)
                            emit_kv_mm(1, xt, k16all, vxs)
                            if fin7 is None:
                                fin6()
                            emit_kv_mm(2, xt, k16all, vxs)
                            emit_kv_mm(3, xt, k16all, vxs)
                            if fin7 is not None:
                                # exps land after all kv copies in the S queue
                                fin6()
                                fin7()
                            emit_k_tails(k16all, ck, sk, kfall)
                            pend_c.append((kfall, vxs, c))
                        else:
                            # last chunk: kv only (q path already done), with
                            # narrow per-subchunk tails so the kv state and
                            # stage C land ASAP.
                            for p in range(NPAIR):
                                nc.sync.dma_start(
                                    out=wo_t[p][:], in_=wo[p * 128 : (p + 1) * 128, :]
                                )
                            for s4 in range(NSUB):
                                emit_kv_mm(s4, xt, k16all, vxs)
                                emit_k_tail_sub(s4, k16all, ck, sk, kfall)
                            pend_c.append((kfall, vxs, c))
                            emit_stage_c(2)  # chunks 6 and 7
                            # build phase-2 stationaries from the kv state
                            # builds alternate V/S per pair: den first (it
                            # gates the first dps matmul), both engines busy
                            for p in range(NPAIR):
                                deng = nc.vector if p % 2 == 0 else None
                                if deng is not None:
                                    deng.tensor_copy(
                                        out=den_f[p][0:64, 0:64],
                                        in_=kvps[p][0:64, 256:257].broadcast_to(
                                            [64, 64]
                                        ),
                                    )
                                    deng.tensor_copy(
                                        out=den_f[p][64:128, 64:128],
                                        in_=kvps[p][64:128, 256:257].broadcast_to(
                                            [64, 64]
                                        ),
                                    )
                                else:
                                    nc.scalar.copy(
                                        out=den_f[p][0:64, 0:64],
                                        in_=kvps[p][0:64, 256:257].broadcast_to(
                                            [64, 64]
                                        ),
                                    )
                                    nc.scalar.copy(
                                        out=den_f[p][64:128, 64:128],
                                        in_=kvps[p][64:128, 256:257].broadcast_to(
                                            [64, 64]
                                        ),
                                    )
                            for p in range(NPAIR):
                                cA = (p % 2) * 128
                                beng = nc.scalar if p % 2 == 0 else None
                                if beng is not None:
                                    beng.copy(
                                        out=bdiag[p][0:64, 0:64],
                                        in_=kvps[p][0:64, cA : cA + 64],
                                    )
                                    beng.copy(
                                        out=bdiag[p][64:128, 64:128],
                                        in_=kvps[p][64:128, cA + 64 : cA + 128],
                                    )
                                else:
                                    nc.vector.tensor_copy(
                                        out=bdiag[p][0:64, 0:64],
                                        in_=kvps[p][0:64, cA : cA + 64],
                                    )
                                    nc.vector.tensor_copy(
                                        out=bdiag[p][64:128, 64:128],
                                        in_=kvps[p][64:128, cA + 64 : cA + 128],
                                    )

            # ---------------- phase 2 (software-pipelined) ----------------
            with tc.tile_pool(name="s2", bufs=2) as s2, tc.tile_pool(
                name="ps2", bufs=1, space="PSUM"
            ) as ps2:

                def emit_recip(cn):
                    """dps matmuls for chunk cn + per-pair reciprocal straight
                    from PSUM (denominators are strictly positive and ~3e5, so
                    the reference's max(x,1e-6) clamp is a no-op)."""
                    rb = s2.tile(
                        [128, NPAIR * CH], FP32, tag="rb", name="rb", bufs=2
                    )
                    for p in range(NPAIR):
                        dps = ps2.tile([128, CH], FP32, tag="dps", bufs=2)
                        nc.tensor.matmul(
                            dps[:],
                            den_f[p][:],
                            qf[:, p * T + cn * CH : p * T + (cn + 1) * CH],
                            start=True,
                            stop=True,
                            skip_group_check=True,
                        )
                        nc.vector.reciprocal_approx_fast(
                            out=rb[:, p * CH : (p + 1) * CH], in_=dps[:]
                        )
                    return rb

                def emit_stage_e(at_l, c_):
                    tsl = slice(c_ * CH, (c_ + 1) * CH)
                    last = c_ == NCH - 1
                    obuf = s2.tile([128, 8 * CH], F16, tag="obuf", name="obuf")
                    outv = outT[:].rearrange("(g p) t -> p g t", p=128)
                    step = 1 if last else 2  # finer DMAs drain the tail faster
                    qi = 0
                    for do in range(8):
                        eps = ps2.tile([128, CH], FP32, tag="eps", bufs=2)
                        for p in range(NPAIR):
                            nc.tensor.matmul(
                                eps[:],
                                wo_t[p][:, do * 128 : (do + 1) * 128],
                                at_l[p][:],
                                start=(p == 0),
                                stop=(p == NPAIR - 1),
                            )
                        if last and do % 2 == 1:
                            nc.vector.tensor_copy(
                                out=obuf[:, do * CH : (do + 1) * CH], in_=eps[:]
                            )
                        else:
                            nc.scalar.copy(
                                out=obuf[:, do * CH : (do + 1) * CH], in_=eps[:]
                            )
                        if do % step == step - 1:
                            g0, g1 = do + 1 - step, do + 1
                            if last:
                                eng = (nc.sync, nc.gpsimd, nc.scalar)[qi % 3]
                            else:
                                eng = (nc.sync, nc.gpsimd)[qi % 2]
                            qi += 1
                            eng.dma_start(
                                out=outv[:, g0:g1, tsl],
                                in_=obuf[:, g0 * CH : g1 * CH].rearrange(
                                    "p (g t) -> p g t", g=g1 - g0
                                ),
                            )

                # warmup: reciprocal chain for chunk 0 (no numerator yet)
                rb_next = emit_recip(0)
                prev = None
                for c in range(NCH):
                    rb = rb_next
                    # numerators + scaling first: the at muls must lead the
                    # next recip chain in the engine queues so stage E is
                    # never starved.
                    at_l = []
                    for p in range(NPAIR):
                        aps = ps2.tile([128, CH], FP32, tag="aps", bufs=4)
                        nc.tensor.matmul(
                            aps[:],
                            bdiag[p][:],
                            qf[:, p * T + c * CH : p * T + (c + 1) * CH],
                            start=True,
                            stop=True,
                        )
                        at = s2.tile([128, CH], F16, tag=f"at{p}", name=f"at{p}")
                        nc.vector.tensor_mul(
                            at[:], aps[:], rb[:, p * CH : (p + 1) * CH]
                        )
                        at_l.append(at)
                    if c + 1 < NCH:
                        rb_next = emit_recip(c + 1)
                    if prev is not None:
                        emit_stage_e(*prev)
                    prev = (at_l, c)
                emit_stage_e(*prev)

    nc.finalize()
    return nc


def _warm_recip_fix(nc):
    return nc


_NC = None


def _get_nc():
    global _NC
    if _NC is None:
        _NC = _build()
    return _NC


def _rope_tables():
    """Interleaved-order rope tables.

    orig head-dim d in [0,64); interleaved position: 2j <- d=j, 2j+1 <- d=j+32.
    rope(x)[d<32] = x[d] cos - x[d+32] sin ; [d>=32] = x[d] cos + x[d-32] sin
    After interleave + XOR-1 partner:
      out[2j]   = x[2j]  * cos_j - partner * sin_j   -> sinS[2j]   = -sin_j
      out[2j+1] = x[2j+1]* cos_j + partner * sin_j   -> sinS[2j+1] = +sin_j
    """
    j = np.arange(32, dtype=np.float64)
    inv_freq = ROPE_BASE ** (-2.0 * j / HD)
    t = np.arange(T, dtype=np.float64)
    ang = t[:, None] * inv_freq[None, :]  # (T, 32)
    cos = np.cos(ang)
    sin = np.sin(ang)
    cos_i = np.empty((T, HD), np.float64)
    sinS_i = np.empty((T, HD), np.float64)
    cos_i[:, 0::2] = cos
    cos_i[:, 1::2] = cos
    sinS_i[:, 0::2] = -sin
    sinS_i[:, 1::2] = sin
    return cos_i, sinS_i


def _perm64():
    p = np.empty(HD, np.int64)
    j = np.arange(32)
    p[2 * j] = j
    p[2 * j + 1] = j + 32
    return p


def _prep_core_inputs(x, W_qkv, W_out):
    """Build the 8 per-core input maps."""
    B = x.shape[0]
    cos_i, sinS_i = _rope_tables()
    perm = _perm64()

    # (d,t)-layout q tables: stacked for the 2 heads of a pair, SCALE folded in
    cosq = np.concatenate([cos_i.T, cos_i.T], axis=0) * SCALE  # (128, T)
    sinq = np.concatenate([sinS_i.T, sinS_i.T], axis=0) * SCALE
    cosq = np.ascontiguousarray(cosq.astype(np.float16))
    sinq = np.ascontiguousarray(sinq.astype(np.float16))
    # (t,d)-layout k tables reshaped (128, 32*64): [p, s*64+d] = tab[s*128+p, d]
    cosk = np.ascontiguousarray(
        cos_i.reshape(32, 128, HD).transpose(1, 0, 2).reshape(128, 32 * HD)
    ).astype(np.float16)
    sink = np.ascontiguousarray(
        sinS_i.reshape(32, 128, HD).transpose(1, 0, 2).reshape(128, 32 * HD)
    ).astype(np.float16)

    in_maps = []
    for core in range(8):
        b, g = divmod(core, 2)
        h0 = g * H_CORE
        qcols = np.concatenate(
            [(h0 + h) * HD + perm for h in range(H_CORE)]
        )  # interleaved q columns
        kcols = 1024 + qcols
        vcols = 2048 + np.arange(h0 * HD, h0 * HD + 512)
        wq_h = np.ascontiguousarray(W_qkv[:, qcols]).astype(np.float16)
        wkv_h = np.ascontiguousarray(
            np.concatenate([W_qkv[:, kcols], W_qkv[:, vcols]], axis=1)
        ).astype(np.float16)
        wo_h = np.ascontiguousarray(W_out[h0 * HD : h0 * HD + 512, :]).astype(
            np.float16
        )
        xT_b = np.ascontiguousarray(x[b].T).astype(np.float16)
        in_maps.append(
            {
                "xT": xT_b,
                "wq": wq_h,
                "wkv": wkv_h,
                "wo": wo_h,
                "cosq": cosq,
                "sinq": sinq,
                "cosk": cosk,
                "sink": sink,
                "ones16": np.ones((128, 4), np.float16),
            }
        )
    return in_maps


def kernel(x, W_qkv, W_out):
    x = np.asarray(x, dtype=np.float32)
    W_qkv = np.asarray(W_qkv, dtype=np.float32)
    W_out = np.asarray(W_out, dtype=np.float32)
    B = x.shape[0]

    nc = _get_nc()
    in_maps = _prep_core_inputs(x, W_qkv, W_out)
    res = run_bass_kernel_spmd(nc, in_maps, core_ids=list(range(8)))

    out = np.empty((B, T, DIM), np.float32)
    for b in range(B):
        acc = res.results[2 * b]["outT"].astype(np.float32) + res.results[
            2 * b + 1
        ]["outT"].astype(np.float32)
        out[b] = acc.T
    return out


# revision 23
# speedup vs baseline: 1.0001x; 1.0001x over previous
"""Trainium2 Bass kernel for nn_LinearSelfAttention (linear attention w/ RoPE,
elu+1 feature map, qkv + out projections).

Sharding: 8 cores = 4 batches x 2 head-groups (8 heads each).
Each core computes, for its (batch b, head-group g):
  qkv slice projection, RoPE, feature maps, per-head kv state (64x64),
  attention output, and a partial out-projection (its heads' rows of W_out).
Host sums the two head-group partials per batch and transposes.

Layout strategy (everything feature-major where matmuls need it):
  - host passes xT = x[b].T (D=1024, T=4096), fp16
  - q is produced in (d, t) layout (W_q stationary, xT moving)
  - k, v are produced in (t, d) layout (xT tiles stationary, W_kv moving)
  - head-dims are interleaved (d, d+32) -> (2j, 2j+1) by permuting W_q/W_k
    columns on the host so the rotate-half partner is the XOR-1 partition
    (reachable by DVE stream_shuffle) / XOR-1 free element.
  - all elementwise stage-B work runs in fp16 (2x DVE mode); PSUM fp32 is
    cast down by one copy per tile, which also releases the PSUM bank early.
  - denominators: a (128,128) block stationary replicates k_sum across 64
    output rows, so the dps matmul emits the reciprocal input already
    broadcast to the (dims, tokens) layout -- no select/broadcast step.
"""

import sys

sys.path.insert(0, "/opt/trn_rl_repo")

import ml_dtypes
import numpy as np

import concourse.bacc as bacc
import concourse.mybir as mybir
from concourse.tile import TileContext
from concourse.bass_utils import run_bass_kernel_spmd

FP32 = mybir.dt.float32
F16 = mybir.dt.float16
AF = mybir.ActivationFunctionType
ALU = mybir.AluOpType

DIM = 1024
T = 4096
HEADS = 16
HD = 64
H_CORE = 8  # heads per core
ROPE_BASE = 500000.0
SCALE = HD**-0.5

NCH = 8  # chunks over T
CH = T // NCH  # 512 tokens per chunk
NSUB = CH // 128  # 4 sub-chunks of 128 tokens
DC = DIM // 128  # 8 contraction tiles
NPAIR = H_CORE // 2  # 4 head pairs (128 dims each)

_SHUF_MASK = [i ^ 1 for i in range(32)]


def _build():
    nc = bacc.Bacc(None, target_bir_lowering=False, debug=False)

    xT = nc.declare_dram_parameter("xT", [DIM, T], F16, isOutput=False)
    wq = nc.declare_dram_parameter("wq", [DIM, 512], F16, isOutput=False)
    wkv = nc.declare_dram_parameter("wkv", [DIM, 1024], F16, isOutput=False)
    wo = nc.declare_dram_parameter("wo", [512, DIM], F16, isOutput=False)
    cosq = nc.declare_dram_parameter("cosq", [128, T], F16, isOutput=False)
    sinq = nc.declare_dram_parameter("sinq", [128, T], F16, isOutput=False)
    cosk = nc.declare_dram_parameter("cosk", [128, 32 * HD], F16, isOutput=False)
    sink = nc.declare_dram_parameter("sink", [128, 32 * HD], F16, isOutput=False)
    ones16 = nc.declare_dram_parameter("ones16", [128, 4], F16, isOutput=False)
    outT = nc.declare_dram_parameter("outT", [DIM, T], F16, isOutput=True)

    with TileContext(nc) as tc, nc.allow_low_precision(
        reason="fp16 tiles feed matmuls; rounding is intended"
    ):
        with tc.tile_pool(name="persist", bufs=1) as persist:
            # all 4 pairs' q features side by side: pair p at cols [p*T, (p+1)*T)
            qf = persist.tile([128, NPAIR * T], F16, tag="qf", name="qf")
            wo_t = [
                persist.tile([128, 1024], F16, tag=f"wo{p}", name=f"wo{p}")
                for p in range(NPAIR)
            ]
            bdiag = [
                persist.tile([128, 128], F16, tag=f"bd{p}", name=f"bd{p}")
                for p in range(NPAIR)
            ]
            den_f = [
                persist.tile([128, 128], F16, tag=f"dn{p}", name=f"dn{p}")
                for p in range(NPAIR)
            ]
            for p in range(NPAIR):
                nc.gpsimd.memset(bdiag[p][:], 0.0)
                nc.gpsimd.memset(den_f[p][:], 0.0)

            with tc.tile_pool(name="pskv", bufs=1, space="PSUM") as pskv:
                kvps = [
                    pskv.tile([128, 258], FP32, tag=f"kv{p}", name=f"kv{p}")
                    for p in range(NPAIR)
                ]

                # ---------------- phase 1 ----------------
                with tc.tile_pool(name="w1", bufs=1) as w1, tc.tile_pool(
                    name="s1", bufs=2
                ) as s1, tc.tile_pool(name="ps1", bufs=1, space="PSUM") as ps1:

                    def load_chunk(c, xt=None, teng=None, xeng=None):
                        teng = teng or nc.gpsimd
                        xeng = xeng or nc.sync
                        tsl = slice(c * CH, (c + 1) * CH)
                        if xt is None:
                            xt = []
                            for d in range(DC):
                                t_ = s1.tile([128, CH], F16, tag=f"x{d}", name=f"x{d}")
                                xeng.dma_start(
                                    out=t_[:], in_=xT[d * 128 : (d + 1) * 128, tsl]
                                )
                                xt.append(t_)
                        cq = s1.tile([128, CH], F16, tag="cq", name="cq")
                        sq = s1.tile([128, CH], F16, tag="sq", name="sq")
                        teng.dma_start(out=cq[:], in_=cosq[:, tsl])
                        teng.dma_start(out=sq[:], in_=sinq[:, tsl])
                        ksl = slice(c * NSUB * HD, (c + 1) * NSUB * HD)
                        ck = s1.tile([128, NSUB * HD], F16, tag="ck", name="ck")
                        sk = s1.tile([128, NSUB * HD], F16, tag="sk", name="sk")
                        teng.dma_start(out=ck[:], in_=cosk[:, ksl])
                        teng.dma_start(out=sk[:], in_=sink[:, ksl])
                        return xt, cq, sq, ck, sk

                    # startup: wq + x chunk0 interleaved on the sync queue
                    # (first matmul only needs wq0+x0); wkv on the scalar
                    # queue so the first kv matmul is never starved.
                    # startup: balance ~4.4MB of initial loads across all 3
                    # DMA queues (~116GB/s each), in consumption order.
                    wq_t = []
                    xt0 = []
                    for d in range(DC):
                        w_ = w1.tile([128, 512], F16, tag=f"wq{d}", name=f"wq{d}")
                        x_ = s1.tile([128, CH], F16, tag=f"x{d}", name=f"x{d}")
                        eng = nc.sync if d < 4 else nc.gpsimd
                        eng.dma_start(out=w_[:], in_=wq[d * 128 : (d + 1) * 128, :])
                        eng.dma_start(out=x_[:], in_=xT[d * 128 : (d + 1) * 128, 0:CH])
                        wq_t.append(w_)
                        xt0.append(x_)
                    wkv_t = []
                    for d in range(DC):
                        t_ = w1.tile([128, 1024], F16, tag=f"wkv{d}", name=f"wkv{d}")
                        eng = (nc.scalar, nc.scalar, nc.scalar, nc.scalar,
                               nc.sync, nc.sync, nc.gpsimd, nc.gpsimd)[d]
                        eng.dma_start(out=t_[:], in_=wkv[d * 128 : (d + 1) * 128, :])
                        wkv_t.append(t_)
                    chunk0 = load_chunk(0, xt0, teng=nc.scalar)

                    pend_c = []  # deferred stage-C batches: (kfall, vxs, c)

                    def emit_stage_c(n=1):
                        for _ in range(min(n, len(pend_c))):
                            kfall_, vxs_, c_ = pend_c.pop(0)
                            for s4 in range(NSUB):
                                s_ = c_ * NSUB + s4
                                vx_ = vxs_[s4]
                                for p in range(NPAIR):
                                    rhs = (
                                        vx_[:, 0:258] if p < 2 else vx_[:, 258:516]
                                    )
                                    nc.tensor.matmul(
                                        kvps[p][:],
                                        kfall_[
                                            :,
                                            s4 * 512 + p * 128 : s4 * 512 + (p + 1) * 128,
                                        ],
                                        rhs,
                                        start=(s_ == 0),
                                        stop=(s_ == NCH * NSUB - 1),
                                        skip_group_check=True,
                                    )

                    def emit_q_mm(ct, xt, q16all):
                        pq = ps1.tile([128, CH], FP32, tag="pq", bufs=2)
                        for d in range(DC):
                            nc.tensor.matmul(
                                pq[:],
                                wq_t[d][:, ct * 128 : (ct + 1) * 128],
                                xt[d][:],
                                start=(d == 0),
                                stop=(d == DC - 1),
                            )
                        # single PSUM reader: releases the bank fast
                        nc.vector.tensor_copy(
                            out=q16all[:, ct * CH : (ct + 1) * CH], in_=pq[:]
                        )

                    def emit_q_tails(q16all, cq, sq, c):
                        tsl = slice(c * CH, (c + 1) * CH)

                        def g3(ap):
                            return ap.rearrange("p (g t) -> p g t", g=NPAIR)

                        cqb = cq[:].unsqueeze(1).broadcast_to([128, NPAIR, CH])
                        sqb = sq[:].unsqueeze(1).broadcast_to([128, NPAIR, CH])
                        sh = s1.tile([128, NPAIR * CH], F16, tag="qA", name="qsh")
                        nc.vector.stream_shuffle(sh[:], q16all[:], _SHUF_MASK)
                        tmp = s1.tile([128, NPAIR * CH], F16, tag="qB", name="qtmp")
                        nc.vector.tensor_mul(g3(tmp[:]), g3(q16all[:]), cqb)
                        u = s1.tile([128, NPAIR * CH], F16, tag="qA", name="qu")
                        nc.vector.tensor_mul(g3(u[:]), g3(sh[:]), sqb)
                        ro = s1.tile([128, NPAIR * CH], F16, tag="qR", name="qro")
                        nc.vector.tensor_add(ro[:], tmp[:], u[:])
                        m = s1.tile([128, NPAIR * CH], F16, tag="qM", name="qm")
                        nc.vector.tensor_scalar_min(m[:], ro[:], 0.0)

                        def finish():
                            e = s1.tile(
                                [128, NPAIR * CH], F16, tag="qB", name="qe"
                            )
                            nc.scalar.activation(e[:], m[:], AF.Exp, scale=1.0)
                            nc.vector.scalar_tensor_tensor(
                                qf[:].rearrange("p (g t) -> p g t", g=NPAIR)[
                                    :, :, tsl
                                ],
                                in0=g3(ro[:]),
                                scalar=0.0,
                                in1=g3(e[:]),
                                op0=ALU.max,
                                op1=ALU.add,
                            )

                        return finish

                    def emit_kv_mm(s4, xt, k16all, vxs):
                        pk = ps1.tile([128, 512], FP32, tag="pk")
                        pv = ps1.tile([128, 512], FP32, tag="pv")
                        for d in range(DC):
                            lhs = xt[d][:, s4 * 128 : (s4 + 1) * 128]
                            nc.tensor.matmul(
                                pk[:],
                                lhs,
                                wkv_t[d][:, 0:512],
                                start=(d == 0),
                                stop=(d == DC - 1),
                                skip_group_check=True,
                            )
                            nc.tensor.matmul(
                                pv[:],
                                lhs,
                                wkv_t[d][:, 512:1024],
                                start=(d == 0),
                                stop=(d == DC - 1),
                                skip_group_check=True,
                            )
                        nc.scalar.copy(
                            out=k16all[:, s4 * 512 : (s4 + 1) * 512], in_=pk[:]
                        )
                        vx = s1.tile([128, 516], F16, tag="vx", name="vx", bufs=12)
                        nc.scalar.copy(
                            out=vx[:].rearrange("p (g c) -> p g c", g=2, c=258)[
                                :, :, 0:256
                            ],
                            in_=pv[:].rearrange("p (g c) -> p g c", g=2, c=256),
                        )
                        nc.gpsimd.dma_start(
                            out=vx[:].rearrange("p (g c) -> p g c", g=2, c=258)[
                                :, :, 256:258
                            ],
                            in_=ones16[:].rearrange("p (g c) -> p g c", g=2),
                        )
                        vxs.append(vx)

                    def emit_k_tails(k16all, ck, sk, kfall):
                        def v4(ap):
                            return ap.rearrange(
                                "p (s h d) -> p s h d", s=NSUB, h=H_CORE
                            )

                        ckb = (
                            ck[:]
                            .rearrange("p (s d) -> p s d", s=NSUB)
                            .unsqueeze(2)
                            .broadcast_to([128, NSUB, H_CORE, HD])
                        )
                        tmpk = s1.tile([128, NSUB * 512], F16, tag="kB", name="ktmp")
                        nc.vector.tensor_mul(v4(tmpk[:]), v4(k16all[:]), ckb)
                        uk = s1.tile([128, NSUB * 512], F16, tag="kC", name="kuk")
                        uk5 = uk[:].rearrange(
                            "p (s h j two) -> p s h j two", s=NSUB, h=H_CORE, j=32
                        )
                        k5 = k16all[:].rearrange(
                            "p (s h j two) -> p s h j two", s=NSUB, h=H_CORE, j=32
                        )
                        sk5 = sk[:].rearrange("p (s j two) -> p s j two", s=NSUB, j=32)
                        for ev in range(2):
                            nc.vector.tensor_mul(
                                uk5[:, :, :, :, ev],
                                k5[:, :, :, :, 1 - ev],
                                sk5[:, :, :, ev : ev + 1]
                                .unsqueeze(2)
                                .broadcast_to([128, NSUB, H_CORE, 32, 1])[
                                    :, :, :, :, 0
                                ],
                            )
                        rok = s1.tile([128, NSUB * 512], F16, tag="kB", name="krok")
                        nc.vector.tensor_add(rok[:], tmpk[:], uk[:])
                        mk = s1.tile([128, NSUB * 512], F16, tag="kC", name="kmk")
                        nc.vector.tensor_scalar_min(mk[:], rok[:], 0.0)
                        ek = s1.tile([128, NSUB * 512], F16, tag="kB", name="kek")
                        nc.scalar.activation(ek[:], mk[:], AF.Exp, scale=1.0)
                        nc.vector.scalar_tensor_tensor(
                            kfall[:],
                            in0=rok[:],
                            scalar=0.0,
                            in1=ek[:],
                            op0=ALU.max,
                            op1=ALU.add,
                        )

                    def emit_k_tail_sub(s4, k16all, ck, sk, kfall):
                        """Narrow (512-wide) k tail for one sub-chunk: short
                        critical chain for the last chunk's kv state."""
                        sl = slice(s4 * 512, (s4 + 1) * 512)
                        ksl = slice(s4 * HD, (s4 + 1) * HD)
                        tmpk = s1.tile([128, 512], F16, tag="kBn", name="ktmpn")
                        nc.vector.tensor_mul(
                            tmpk[:].rearrange("p (h d) -> p h d", h=H_CORE),
                            k16all[:, sl].rearrange("p (h d) -> p h d", h=H_CORE),
                            ck[:, ksl].unsqueeze(1).broadcast_to(
                                [128, H_CORE, HD]
                            ),
                        )
                        uk = s1.tile([128, 512], F16, tag="kCn", name="kukn")
                        uk4 = uk[:].rearrange(
                            "p (h j two) -> p h j two", h=H_CORE, j=32
                        )
                        k4 = k16all[:, sl].rearrange(
                            "p (h j two) -> p h j two", h=H_CORE, j=32
                        )
                        sk4 = sk[:, ksl].rearrange("p (j two) -> p j two", j=32)
                        for ev in range(2):
                            nc.vector.tensor_mul(
                                uk4[:, :, :, ev],
                                k4[:, :, :, 1 - ev],
                                sk4[:, :, ev : ev + 1]
                                .unsqueeze(1)
                                .broadcast_to([128, H_CORE, 32, 1])[:, :, :, 0],
                            )
                        rok = s1.tile([128, 512], F16, tag="kBn", name="krokn")
                        nc.vector.tensor_add(rok[:], tmpk[:], uk[:])
                        mk = s1.tile([128, 512], F16, tag="kCn", name="kmkn")
                        nc.vector.tensor_scalar_min(mk[:], rok[:], 0.0)
                        ek = s1.tile([128, 512], F16, tag="kBn", name="kekn")
                        nc.scalar.activation(ek[:], mk[:], AF.Exp, scale=1.0)
                        nc.vector.scalar_tensor_tensor(
                            kfall[:, sl],
                            in0=rok[:],
                            scalar=0.0,
                            in1=ek[:],
                            op0=ALU.max,
                            op1=ALU.add,
                        )

                    nxt = None  # prefetched loads for the last chunk
                    for c in range(NCH):
                        if c == 0:
                            xt, cq, sq, ck, sk = chunk0
                        elif c == NCH - 1:
                            xt, cq, sq, ck, sk = nxt
                        else:
                            # chunk 1's x rides the scalar queue, which drains
                            # its startup share ~4us before sync does
                            xt, cq, sq, ck, sk = load_chunk(
                                c, xeng=nc.scalar if c == 1 else None
                            )
                        if c >= 2:
                            emit_stage_c(1)
                        q16all = s1.tile(
                            [128, NPAIR * CH], F16, tag="q16", name="q16"
                        )
                        k16all = s1.tile(
                            [128, NSUB * 512], F16, tag="k16", name="k16"
                        )
                        kfall = s1.tile(
                            [128, NSUB * 512], F16, tag="kf", name="kf", bufs=3
                        )
                        vxs = []
                        if c < NCH - 1:
                            for ct in range(NPAIR):
                                emit_q_mm(ct, xt, q16all)
                            emit_kv_mm(0, xt, k16all, vxs)
                            fin7 = None
                            if c == NCH - 2:
                                # run the LAST chunk's whole q path now (mid
                                # chunk 6) so its Vector work drains before
                                # the phase transition; its PSUM-freeing
                                # copies must precede the wide tail chains.
                                nxt = load_chunk(NCH - 1)
                                q16b = s1.tile(
                                    [128, NPAIR * CH], F16, tag="q16", name="q16b"
                                )
                                for ct in range(NPAIR):
                                    emit_q_mm(ct, nxt[0], q16b)
                                fin6 = emit_q_tails(q16all, cq, sq, c)
                                fin7 = emit_q_tails(q16b, nxt[1], nxt[2], NCH - 1)
                            else:
                                fin6 = emit_q_tails(q16all, cq, sq, c)
                            emit_kv_mm(1, xt, k16all, vxs)
                            if fin7 is None:
                                fin6()
                            emit_kv_mm(2, xt, k16all, vxs)
                            emit_kv_mm(3, xt, k16all, vxs)
                            if fin7 is not None:
                                # exps land after all kv copies in the S queue
                                fin6()
                                fin7()
                            emit_k_tails(k16all, ck, sk, kfall)
                            pend_c.append((kfall, vxs, c))
                        else:
                            # last chunk: kv only (q path already done), with
                            # narrow per-subchunk tails so the kv state and
                            # stage C land ASAP.
                            for p in range(NPAIR):
                                nc.sync.dma_start(
                                    out=wo_t[p][:], in_=wo[p * 128 : (p + 1) * 128, :]
                                )
                            for s4 in range(NSUB):
                                emit_kv_mm(s4, xt, k16all, vxs)
                                emit_k_tail_sub(s4, k16all, ck, sk, kfall)
                            pend_c.append((kfall, vxs, c))
                            emit_stage_c(2)  # chunks 6 and 7
                            # build phase-2 stationaries from the kv state
                            # builds alternate V/S per pair: den first (it
                            # gates the first dps matmul), both engines busy
                            for p in range(NPAIR):
                                deng = nc.vector if p % 2 == 0 else None
                                if deng is not None:
                                    deng.tensor_copy(
                                        out=den_f[p][0:64, 0:64],
                                        in_=kvps[p][0:64, 256:257].broadcast_to(
                                            [64, 64]
                                        ),
                                    )
                                    deng.tensor_copy(
                                        out=den_f[p][64:128, 64:128],
                                        in_=kvps[p][64:128, 256:257].broadcast_to(
                                            [64, 64]
                                        ),
                                    )
                                else:
                                    nc.scalar.copy(
                                        out=den_f[p][0:64, 0:64],
                                        in_=kvps[p][0:64, 256:257].broadcast_to(
                                            [64, 64]
                                        ),
                                    )
                                    nc.scalar.copy(
                                        out=den_f[p][64:128, 64:128],
                                        in_=kvps[p][64:128, 256:257].broadcast_to(
                                            [64, 64]
                                        ),
                                    )
                            for p in range(NPAIR):
                                cA = (p % 2) * 128
                                beng = nc.scalar if p % 2 == 0 else None
                                if beng is not None:
                                    beng.copy(
                                        out=bdiag[p][0:64, 0:64],
                                        in_=kvps[p][0:64, cA : cA + 64],
                                    )
                                    beng.copy(
                                        out=bdiag[p][64:128, 64:128],
                                        in_=kvps[p][64:128, cA + 64 : cA + 128],
                                    )
                                else:
                                    nc.vector.tensor_copy(
                                        out=bdiag[p][0:64, 0:64],
                                        in_=kvps[p][0:64, cA : cA + 64],
                                    )
                                    nc.vector.tensor_copy(
                                        out=bdiag[p][64:128, 64:128],
                                        in_=kvps[p][64:128, cA + 64 : cA + 128],
                                    )

            # ---------------- phase 2 (software-pipelined) ----------------
            with tc.tile_pool(name="s2", bufs=2) as s2, tc.tile_pool(
                name="ps2", bufs=1, space="PSUM"
            ) as ps2:

                def emit_recip(cn):
                    """dps matmuls for chunk cn + per-pair reciprocal straight
                    from PSUM (denominators are strictly positive and ~3e5, so
                    the reference's max(x,1e-6) clamp is a no-op)."""
                    rb = s2.tile(
                        [128, NPAIR * CH], FP32, tag="rb", name="rb", bufs=2
                    )
                    for p in range(NPAIR):
                        dps = ps2.tile([128, CH], FP32, tag="dps", bufs=2)
                        nc.tensor.matmul(
                            dps[:],
                            den_f[p][:],
                            qf[:, p * T + cn * CH : p * T + (cn + 1) * CH],
                            start=True,
                            stop=True,
                            skip_group_check=True,
                        )
                        nc.vector.reciprocal_approx_fast(
                            out=rb[:, p * CH : (p + 1) * CH], in_=dps[:]
                        )
                    return rb

                def emit_stage_e(at_l, c_):
                    tsl = slice(c_ * CH, (c_ + 1) * CH)
                    last = c_ == NCH - 1
                    obuf = s2.tile([128, 8 * CH], F16, tag="obuf", name="obuf")
                    outv = outT[:].rearrange("(g p) t -> p g t", p=128)
                    step = 1 if last else 2  # finer DMAs drain the tail faster
                    qi = 0
                    for do in range(8):
                        eps = ps2.tile([128, CH], FP32, tag="eps", bufs=2)
                        for p in range(NPAIR):
                            nc.tensor.matmul(
                                eps[:],
                                wo_t[p][:, do * 128 : (do + 1) * 128],
                                at_l[p][:],
                                start=(p == 0),
                                stop=(p == NPAIR - 1),
                            )
                        if last and do % 2 == 1:
                            nc.vector.tensor_copy(
                                out=obuf[:, do * CH : (do + 1) * CH], in_=eps[:]
                            )
                        else:
                            nc.scalar.copy(
                                out=obuf[:, do * CH : (do + 1) * CH], in_=eps[:]
                            )
                        if do % step == step - 1:
                            g0, g1 = do + 1 - step, do + 1
                            if last:
                                eng = (nc.sync, nc.gpsimd, nc.scalar)[qi % 3]
                            else:
                                eng = (nc.sync, nc.gpsimd)[qi % 2]
                            qi += 1
                            eng.dma_start(
                                out=outv[:, g0:g1, tsl],
                                in_=obuf[:, g0 * CH : g1 * CH].rearrange(
                                    "p (g t) -> p g t", g=g1 - g0
                                ),
                            )

                # warmup: reciprocal chain for chunk 0 (no numerator yet)
                rb_next = emit_recip(0)
                prev = None
                for c in range(NCH):
                    rb = rb_next
                    # numerators + scaling first: the at muls must lead the
                    # next recip chain in the engine queues so stage E is
                    # never starved.
                    at_l = []
                    for p in range(NPAIR):
                        aps = ps2.tile([128, CH], FP32, tag="aps", bufs=4)
                        nc.tensor.matmul(
                            aps[:],
                            bdiag[p][:],
                            qf[:, p * T + c * CH : p * T + (c + 1) * CH],
                            start=True,
                            stop=True,
                        )
                        at = s2.tile([128, CH], F16, tag=f"at{p}", name=f"at{p}")
                        nc.vector.tensor_mul(
                            at[:], aps[:], rb[:, p * CH : (p + 1) * CH]
                        )
                        at_l.append(at)
                    if c + 1 < NCH:
                        rb_next = emit_recip(c + 1)
                    if prev is not None:
                        emit_stage_e(*prev)
                    prev = (at_l, c)
                emit_stage_e(*prev)

    nc.finalize()
    return nc


def _warm_recip_fix(nc):
    return nc


_NC = None


def _get_nc():
    global _NC
    if _NC is None:
        _NC = _build()
    return _NC


def _rope_tables():
    """Interleaved-order rope tables.

    orig head-dim d in [0,64); interleaved position: 2j <- d=j, 2j+1 <- d=j+32.
    rope(x)[d<32] = x[d] cos - x[d+32] sin ; [d>=32] = x[d] cos + x[d-32] sin
    After interleave + XOR-1 partner:
      out[2j]   = x[2j]  * cos_j - partner * sin_j   -> sinS[2j]   = -sin_j
      out[2j+1] = x[2j+1]* cos_j + partner * sin_j   -> sinS[2j+1] = +sin_j
    """
    j = np.arange(32, dtype=np.float64)
    inv_freq = ROPE_BASE ** (-2.0 * j / HD)
    t = np.arange(T, dtype=np.float64)
    ang = t[:, None] * inv_freq[None, :]  # (T, 32)
    cos = np.cos(ang)
    sin = np.sin(ang)
    cos_i = np.empty((T, HD), np.float64)
    sinS_i = np.empty((T, HD), np.float64)
    cos_i[:, 0::2] = cos
    cos_i[:, 1::2] = cos
    sinS_i[:, 0::2] = -sin
    sinS_i[:, 1::2] = sin
    return cos_i, sinS_i


def _perm64():
    p = np.empty(HD, np.int64)
    j = np.arange(32)
    p[2 * j] = j
    p[2 * j + 1] = j + 32
    return p


def _prep_core_inputs(x, W_qkv, W_out):
    """Build the 8 per-core input maps."""
    B = x.shape[0]
    cos_i, sinS_i = _rope_tables()
    perm = _perm64()

    # (d,t)-layout q tables: stacked for the 2 heads of a pair, SCALE folded in
    cosq = np.concatenate([cos_i.T, cos_i.T], axis=0) * SCALE  # (128, T)
    sinq = np.concatenate([sinS_i.T, sinS_i.T], axis=0) * SCALE
    cosq = np.ascontiguousarray(cosq.astype(np.float16))
    sinq = np.ascontiguousarray(sinq.astype(np.float16))
    # (t,d)-layout k tables reshaped (128, 32*64): [p, s*64+d] = tab[s*128+p, d]
    cosk = np.ascontiguousarray(
        cos_i.reshape(32, 128, HD).transpose(1, 0, 2).reshape(128, 32 * HD)
    ).astype(np.float16)
    sink = np.ascontiguousarray(
        sinS_i.reshape(32, 128, HD).transpose(1, 0, 2).reshape(128, 32 * HD)
    ).astype(np.float16)

    in_maps = []
    for core in range(8):
        b, g = divmod(core, 2)
        h0 = g * H_CORE
        qcols = np.concatenate(
            [(h0 + h) * HD + perm for h in range(H_CORE)]
        )  # interleaved q columns
        kcols = 1024 + qcols
        vcols = 2048 + np.arange(h0 * HD, h0 * HD + 512)
        wq_h = np.ascontiguousarray(W_qkv[:, qcols]).astype(np.float16)
        wkv_h = np.ascontiguousarray(
            np.concatenate([W_qkv[:, kcols], W_qkv[:, vcols]], axis=1)
        ).astype(np.float16)
        wo_h = np.ascontiguousarray(W_out[h0 * HD : h0 * HD + 512, :]).astype(
            np.float16
        )
        xT_b = np.ascontiguousarray(x[b].T).astype(np.float16)
        in_maps.append(
            {
                "xT": xT_b,
                "wq": wq_h,
                "wkv": wkv_h,
                "wo": wo_h,
                "cosq": cosq,
                "sinq": sinq,
                "cosk": cosk,
                "sink": sink,
                "ones16": np.ones((128, 4), np.float16),
            }
        )
    return in_maps


def kernel(x, W_qkv, W_out):
    x = np.asarray(x, dtype=np.float32)
    W_qkv = np.asarray(W_qkv, dtype=np.float32)
    W_out = np.asarray(W_out, dtype=np.float32)
    B = x.shape[0]

    nc = _get_nc()
    in_maps = _prep_core_inputs(x, W_qkv, W_out)
    res = run_bass_kernel_spmd(nc, in_maps, core_ids=list(range(8)))

    out = np.empty((B, T, DIM), np.float32)
    for b in range(B):
        acc = res.results[2 * b]["outT"].astype(np.float32) + res.results[
            2 * b + 1
        ]["outT"].astype(np.float32)
        out[b] = acc.T
    return out


# revision 24
# speedup vs baseline: 1.0090x; 1.0088x over previous
"""Trainium2 Bass kernel for nn_LinearSelfAttention (linear attention w/ RoPE,
elu+1 feature map, qkv + out projections).

Sharding: 8 cores = 4 batches x 2 head-groups (8 heads each).
Each core computes, for its (batch b, head-group g):
  qkv slice projection, RoPE, feature maps, per-head kv state (64x64),
  attention output, and a partial out-projection (its heads' rows of W_out).
Host sums the two head-group partials per batch and transposes.

Layout strategy (everything feature-major where matmuls need it):
  - host passes xT = x[b].T (D=1024, T=4096), fp16
  - q is produced in (d, t) layout (W_q stationary, xT moving)
  - k, v are produced in (t, d) layout (xT tiles stationary, W_kv moving)
  - head-dims are interleaved (d, d+32) -> (2j, 2j+1) by permuting W_q/W_k
    columns on the host so the rotate-half partner is the XOR-1 partition
    (reachable by DVE stream_shuffle) / XOR-1 free element.
  - all elementwise stage-B work runs in fp16 (2x DVE mode); PSUM fp32 is
    cast down by one copy per tile, which also releases the PSUM bank early.
  - denominators: a (128,128) block stationary replicates k_sum across 64
    output rows, so the dps matmul emits the reciprocal input already
    broadcast to the (dims, tokens) layout -- no select/broadcast step.
"""

import sys

sys.path.insert(0, "/opt/trn_rl_repo")

import ml_dtypes
import numpy as np

import concourse.bacc as bacc
import concourse.mybir as mybir
from concourse.tile import TileContext
from concourse.bass_utils import run_bass_kernel_spmd

FP32 = mybir.dt.float32
F16 = mybir.dt.float16
AF = mybir.ActivationFunctionType
ALU = mybir.AluOpType

DIM = 1024
T = 4096
HEADS = 16
HD = 64
H_CORE = 8  # heads per core
ROPE_BASE = 500000.0
SCALE = HD**-0.5

NCH = 8  # chunks over T
CH = T // NCH  # 512 tokens per chunk
NSUB = CH // 128  # 4 sub-chunks of 128 tokens
DC = DIM // 128  # 8 contraction tiles
NPAIR = H_CORE // 2  # 4 head pairs (128 dims each)

_SHUF_MASK = [i ^ 1 for i in range(32)]


def _build():
    nc = bacc.Bacc(None, target_bir_lowering=False, debug=False)

    xT = nc.declare_dram_parameter("xT", [DIM, T], F16, isOutput=False)
    wq = nc.declare_dram_parameter("wq", [DIM, 512], F16, isOutput=False)
    wkv = nc.declare_dram_parameter("wkv", [DIM, 1024], F16, isOutput=False)
    wo = nc.declare_dram_parameter("wo", [512, DIM], F16, isOutput=False)
    cosq = nc.declare_dram_parameter("cosq", [128, T], F16, isOutput=False)
    sinq = nc.declare_dram_parameter("sinq", [128, T], F16, isOutput=False)
    cosk = nc.declare_dram_parameter("cosk", [128, 32 * HD], F16, isOutput=False)
    sink = nc.declare_dram_parameter("sink", [128, 32 * HD], F16, isOutput=False)
    ones16 = nc.declare_dram_parameter("ones16", [128, 4], F16, isOutput=False)
    outT = nc.declare_dram_parameter("outT", [DIM, T], F16, isOutput=True)

    with TileContext(nc) as tc, nc.allow_low_precision(
        reason="fp16 tiles feed matmuls; rounding is intended"
    ):
        with tc.tile_pool(name="persist", bufs=1) as persist:
            # all 4 pairs' q features side by side: pair p at cols [p*T, (p+1)*T)
            qf = persist.tile([128, NPAIR * T], F16, tag="qf", name="qf")
            wo_t = [
                persist.tile([128, 1024], F16, tag=f"wo{p}", name=f"wo{p}")
                for p in range(NPAIR)
            ]
            bdiag = [
                persist.tile([128, 128], F16, tag=f"bd{p}", name=f"bd{p}")
                for p in range(NPAIR)
            ]
            den_f = [
                persist.tile([128, 128], F16, tag=f"dn{p}", name=f"dn{p}")
                for p in range(NPAIR)
            ]
            for p in range(NPAIR):
                nc.gpsimd.memset(bdiag[p][:], 0.0)
                nc.gpsimd.memset(den_f[p][:], 0.0)

            with tc.tile_pool(name="pskv", bufs=1, space="PSUM") as pskv:
                kvps = [
                    pskv.tile([128, 258], FP32, tag=f"kv{p}", name=f"kv{p}")
                    for p in range(NPAIR)
                ]

                # ---------------- phase 1 ----------------
                with tc.tile_pool(name="w1", bufs=1) as w1, tc.tile_pool(
                    name="s1", bufs=2
                ) as s1, tc.tile_pool(name="ps1", bufs=1, space="PSUM") as ps1:

                    def load_chunk(c, xt=None, teng=None, xeng=None):
                        teng = teng or nc.gpsimd
                        xeng = xeng or nc.sync
                        tsl = slice(c * CH, (c + 1) * CH)
                        if xt is None:
                            xt = []
                            for d in range(DC):
                                t_ = s1.tile([128, CH], F16, tag=f"x{d}", name=f"x{d}")
                                xeng.dma_start(
                                    out=t_[:], in_=xT[d * 128 : (d + 1) * 128, tsl]
                                )
                                xt.append(t_)
                        cq = s1.tile([128, CH], F16, tag="cq", name="cq")
                        sq = s1.tile([128, CH], F16, tag="sq", name="sq")
                        teng.dma_start(out=cq[:], in_=cosq[:, tsl])
                        teng.dma_start(out=sq[:], in_=sinq[:, tsl])
                        ksl = slice(c * NSUB * HD, (c + 1) * NSUB * HD)
                        ck = s1.tile([128, NSUB * HD], F16, tag="ck", name="ck")
                        sk = s1.tile([128, NSUB * HD], F16, tag="sk", name="sk")
                        teng.dma_start(out=ck[:], in_=cosk[:, ksl])
                        teng.dma_start(out=sk[:], in_=sink[:, ksl])
                        return xt, cq, sq, ck, sk

                    # startup: wq + x chunk0 interleaved on the sync queue
                    # (first matmul only needs wq0+x0); wkv on the scalar
                    # queue so the first kv matmul is never starved.
                    # startup: balance ~4.4MB of initial loads across all 3
                    # DMA queues (~116GB/s each), in consumption order.
                    wq_t = []
                    xt0 = []
                    for d in range(DC):
                        w_ = w1.tile([128, 512], F16, tag=f"wq{d}", name=f"wq{d}")
                        x_ = s1.tile([128, CH], F16, tag=f"x{d}", name=f"x{d}")
                        eng = nc.sync if d < 4 else nc.gpsimd
                        eng.dma_start(out=w_[:], in_=wq[d * 128 : (d + 1) * 128, :])
                        eng.dma_start(out=x_[:], in_=xT[d * 128 : (d + 1) * 128, 0:CH])
                        wq_t.append(w_)
                        xt0.append(x_)
                    wkv_t = []
                    for d in range(DC):
                        t_ = w1.tile([128, 1024], F16, tag=f"wkv{d}", name=f"wkv{d}")
                        eng = (nc.scalar, nc.scalar, nc.scalar, nc.scalar,
                               nc.sync, nc.sync, nc.gpsimd, nc.gpsimd)[d]
                        eng.dma_start(out=t_[:], in_=wkv[d * 128 : (d + 1) * 128, :])
                        wkv_t.append(t_)
                    chunk0 = load_chunk(0, xt0, teng=nc.scalar)

                    pend_c = []  # deferred stage-C batches: (kfall, vxs, c)

                    def emit_stage_c(n=1):
                        for _ in range(min(n, len(pend_c))):
                            kfall_, vxs_, c_ = pend_c.pop(0)
                            for s4 in range(NSUB):
                                s_ = c_ * NSUB + s4
                                vx_ = vxs_[s4]
                                for p in range(NPAIR):
                                    rhs = (
                                        vx_[:, 0:258] if p < 2 else vx_[:, 258:516]
                                    )
                                    nc.tensor.matmul(
                                        kvps[p][:],
                                        kfall_[
                                            :,
                                            s4 * 512 + p * 128 : s4 * 512 + (p + 1) * 128,
                                        ],
                                        rhs,
                                        start=(s_ == 0),
                                        stop=(s_ == NCH * NSUB - 1),
                                        skip_group_check=True,
                                    )

                    def emit_q_mm(ct, xt, q16all):
                        pq = ps1.tile([128, CH], FP32, tag="pq", bufs=2)
                        for d in range(DC):
                            nc.tensor.matmul(
                                pq[:],
                                wq_t[d][:, ct * 128 : (ct + 1) * 128],
                                xt[d][:],
                                start=(d == 0),
                                stop=(d == DC - 1),
                            )
                        # single PSUM reader: releases the bank fast
                        nc.vector.tensor_copy(
                            out=q16all[:, ct * CH : (ct + 1) * CH], in_=pq[:]
                        )

                    def emit_q_tails(q16all, cq, sq, c):
                        tsl = slice(c * CH, (c + 1) * CH)

                        def g3(ap):
                            return ap.rearrange("p (g t) -> p g t", g=NPAIR)

                        cqb = cq[:].unsqueeze(1).broadcast_to([128, NPAIR, CH])
                        sqb = sq[:].unsqueeze(1).broadcast_to([128, NPAIR, CH])
                        sh = s1.tile([128, NPAIR * CH], F16, tag="qA", name="qsh")
                        nc.vector.stream_shuffle(sh[:], q16all[:], _SHUF_MASK)
                        tmp = s1.tile([128, NPAIR * CH], F16, tag="qB", name="qtmp")
                        nc.vector.tensor_mul(g3(tmp[:]), g3(q16all[:]), cqb)
                        u = s1.tile([128, NPAIR * CH], F16, tag="qA", name="qu")
                        nc.vector.tensor_mul(g3(u[:]), g3(sh[:]), sqb)
                        ro = s1.tile([128, NPAIR * CH], F16, tag="qR", name="qro")
                        nc.vector.tensor_add(ro[:], tmp[:], u[:])
                        m = s1.tile([128, NPAIR * CH], F16, tag="qM", name="qm")
                        nc.vector.tensor_scalar_min(m[:], ro[:], 0.0)

                        def finish():
                            e = s1.tile(
                                [128, NPAIR * CH], F16, tag="qB", name="qe"
                            )
                            nc.scalar.activation(e[:], m[:], AF.Exp, scale=1.0)
                            nc.vector.scalar_tensor_tensor(
                                qf[:].rearrange("p (g t) -> p g t", g=NPAIR)[
                                    :, :, tsl
                                ],
                                in0=g3(ro[:]),
                                scalar=0.0,
                                in1=g3(e[:]),
                                op0=ALU.max,
                                op1=ALU.add,
                            )

                        return finish

                    def emit_kv_mm(s4, xt, k16all, vxs):
                        pk = ps1.tile([128, 512], FP32, tag="pk")
                        pv = ps1.tile([128, 512], FP32, tag="pv")
                        for d in range(DC):
                            lhs = xt[d][:, s4 * 128 : (s4 + 1) * 128]
                            nc.tensor.matmul(
                                pk[:],
                                lhs,
                                wkv_t[d][:, 0:512],
                                start=(d == 0),
                                stop=(d == DC - 1),
                                skip_group_check=True,
                            )
                            nc.tensor.matmul(
                                pv[:],
                                lhs,
                                wkv_t[d][:, 512:1024],
                                start=(d == 0),
                                stop=(d == DC - 1),
                                skip_group_check=True,
                            )
                        nc.scalar.copy(
                            out=k16all[:, s4 * 512 : (s4 + 1) * 512], in_=pk[:]
                        )
                        vx = s1.tile([128, 516], F16, tag="vx", name="vx", bufs=12)
                        nc.scalar.copy(
                            out=vx[:].rearrange("p (g c) -> p g c", g=2, c=258)[
                                :, :, 0:256
                            ],
                            in_=pv[:].rearrange("p (g c) -> p g c", g=2, c=256),
                        )
                        nc.gpsimd.dma_start(
                            out=vx[:].rearrange("p (g c) -> p g c", g=2, c=258)[
                                :, :, 256:258
                            ],
                            in_=ones16[:].rearrange("p (g c) -> p g c", g=2),
                        )
                        vxs.append(vx)

                    def emit_k_tails(k16all, ck, sk, kfall):
                        def v4(ap):
                            return ap.rearrange(
                                "p (s h d) -> p s h d", s=NSUB, h=H_CORE
                            )

                        ckb = (
                            ck[:]
                            .rearrange("p (s d) -> p s d", s=NSUB)
                            .unsqueeze(2)
                            .broadcast_to([128, NSUB, H_CORE, HD])
                        )
                        tmpk = s1.tile([128, NSUB * 512], F16, tag="kB", name="ktmp")
                        nc.vector.tensor_mul(v4(tmpk[:]), v4(k16all[:]), ckb)
                        uk = s1.tile([128, NSUB * 512], F16, tag="kC", name="kuk")
                        uk5 = uk[:].rearrange(
                            "p (s h j two) -> p s h j two", s=NSUB, h=H_CORE, j=32
                        )
                        k5 = k16all[:].rearrange(
                            "p (s h j two) -> p s h j two", s=NSUB, h=H_CORE, j=32
                        )
                        sk5 = sk[:].rearrange("p (s j two) -> p s j two", s=NSUB, j=32)
                        for ev in range(2):
                            nc.vector.tensor_mul(
                                uk5[:, :, :, :, ev],
                                k5[:, :, :, :, 1 - ev],
                                sk5[:, :, :, ev : ev + 1]
                                .unsqueeze(2)
                                .broadcast_to([128, NSUB, H_CORE, 32, 1])[
                                    :, :, :, :, 0
                                ],
                            )
                        rok = s1.tile([128, NSUB * 512], F16, tag="kB", name="krok")
                        nc.vector.tensor_add(rok[:], tmpk[:], uk[:])
                        mk = s1.tile([128, NSUB * 512], F16, tag="kC", name="kmk")
                        nc.vector.tensor_scalar_min(mk[:], rok[:], 0.0)
                        ek = s1.tile([128, NSUB * 512], F16, tag="kB", name="kek")
                        nc.scalar.activation(ek[:], mk[:], AF.Exp, scale=1.0)
                        nc.vector.scalar_tensor_tensor(
                            kfall[:],
                            in0=rok[:],
                            scalar=0.0,
                            in1=ek[:],
                            op0=ALU.max,
                            op1=ALU.add,
                        )

                    def emit_k_tail_sub(s4, k16all, ck, sk, kfall):
                        """Narrow (512-wide) k tail for one sub-chunk: short
                        critical chain for the last chunk's kv state."""
                        sl = slice(s4 * 512, (s4 + 1) * 512)
                        ksl = slice(s4 * HD, (s4 + 1) * HD)
                        tmpk = s1.tile([128, 512], F16, tag="kBn", name="ktmpn")
                        nc.vector.tensor_mul(
                            tmpk[:].rearrange("p (h d) -> p h d", h=H_CORE),
                            k16all[:, sl].rearrange("p (h d) -> p h d", h=H_CORE),
                            ck[:, ksl].unsqueeze(1).broadcast_to(
                                [128, H_CORE, HD]
                            ),
                        )
                        uk = s1.tile([128, 512], F16, tag="kCn", name="kukn")
                        uk4 = uk[:].rearrange(
                            "p (h j two) -> p h j two", h=H_CORE, j=32
                        )
                        k4 = k16all[:, sl].rearrange(
                            "p (h j two) -> p h j two", h=H_CORE, j=32
                        )
                        sk4 = sk[:, ksl].rearrange("p (j two) -> p j two", j=32)
                        for ev in range(2):
                            nc.vector.tensor_mul(
                                uk4[:, :, :, ev],
                                k4[:, :, :, 1 - ev],
                                sk4[:, :, ev : ev + 1]
                                .unsqueeze(1)
                                .broadcast_to([128, H_CORE, 32, 1])[:, :, :, 0],
                            )
                        rok = s1.tile([128, 512], F16, tag="kBn", name="krokn")
                        nc.vector.tensor_add(rok[:], tmpk[:], uk[:])
                        mk = s1.tile([128, 512], F16, tag="kCn", name="kmkn")
                        nc.vector.tensor_scalar_min(mk[:], rok[:], 0.0)
                        ek = s1.tile([128, 512], F16, tag="kBn", name="kekn")
                        nc.scalar.activation(ek[:], mk[:], AF.Exp, scale=1.0)
                        nc.vector.scalar_tensor_tensor(
                            kfall[:, sl],
                            in0=rok[:],
                            scalar=0.0,
                            in1=ek[:],
                            op0=ALU.max,
                            op1=ALU.add,
                        )

                    nxt = None  # prefetched loads for the last chunk
                    for c in range(NCH):
                        if c == 0:
                            xt, cq, sq, ck, sk = chunk0
                        elif c == NCH - 1:
                            xt, cq, sq, ck, sk = nxt
                        else:
                            # chunk 1's x rides the scalar queue, which drains
                            # its startup share ~4us before sync does
                            xt, cq, sq, ck, sk = load_chunk(
                                c, xeng=nc.scalar if c == 1 else None
                            )
                        if c >= 2:
                            emit_stage_c(1)
                        q16all = s1.tile(
                            [128, NPAIR * CH], F16, tag="q16", name="q16"
                        )
                        k16all = s1.tile(
                            [128, NSUB * 512], F16, tag="k16", name="k16"
                        )
                        kfall = s1.tile(
                            [128, NSUB * 512], F16, tag="kf", name="kf", bufs=3
                        )
                        vxs = []
                        if c < NCH - 1:
                            for ct in range(NPAIR):
                                emit_q_mm(ct, xt, q16all)
                            emit_kv_mm(0, xt, k16all, vxs)
                            fin7 = None
                            if c == NCH - 2:
                                # run the LAST chunk's whole q path now (mid
                                # chunk 6) so its Vector work drains before
                                # the phase transition; its PSUM-freeing
                                # copies must precede the wide tail chains.
                                nxt = load_chunk(NCH - 1)
                                q16b = s1.tile(
                                    [128, NPAIR * CH], F16, tag="q16", name="q16b"
                                )
                                for ct in range(NPAIR):
                                    emit_q_mm(ct, nxt[0], q16b)
                                fin6 = emit_q_tails(q16all, cq, sq, c)
                                fin7 = emit_q_tails(q16b, nxt[1], nxt[2], NCH - 1)
                            else:
                                fin6 = emit_q_tails(q16all, cq, sq, c)
                            emit_kv_mm(1, xt, k16all, vxs)
                            if fin7 is None:
                                fin6()
                            emit_kv_mm(2, xt, k16all, vxs)
                            emit_kv_mm(3, xt, k16all, vxs)
                            if fin7 is not None:
                                # exps land after all kv copies in the S queue
                                fin6()
                                fin7()
                            emit_k_tails(k16all, ck, sk, kfall)
                            pend_c.append((kfall, vxs, c))
                        else:
                            # last chunk: kv only (q path already done), with
                            # narrow per-subchunk tails so the kv state and
                            # stage C land ASAP.
                            for p in range(NPAIR):
                                nc.sync.dma_start(
                                    out=wo_t[p][:], in_=wo[p * 128 : (p + 1) * 128, :]
                                )
                            for s4 in range(NSUB):
                                emit_kv_mm(s4, xt, k16all, vxs)
                                emit_k_tail_sub(s4, k16all, ck, sk, kfall)
                            pend_c.append((kfall, vxs, c))
                            emit_stage_c(2)  # chunks 6 and 7
                            # build phase-2 stationaries from the kv state
                            # builds alternate V/S per pair: den first (it
                            # gates the first dps matmul), both engines busy
                            for p in range(NPAIR):
                                deng = nc.vector if p % 2 == 0 else None
                                if deng is not None:
                                    deng.tensor_copy(
                                        out=den_f[p][0:64, 0:64],
                                        in_=kvps[p][0:64, 256:257].broadcast_to(
                                            [64, 64]
                                        ),
                                    )
                                    deng.tensor_copy(
                                        out=den_f[p][64:128, 64:128],
                                        in_=kvps[p][64:128, 256:257].broadcast_to(
                                            [64, 64]
                                        ),
                                    )
                                else:
                                    nc.scalar.copy(
                                        out=den_f[p][0:64, 0:64],
                                        in_=kvps[p][0:64, 256:257].broadcast_to(
                                            [64, 64]
                                        ),
                                    )
                                    nc.scalar.copy(
                                        out=den_f[p][64:128, 64:128],
                                        in_=kvps[p][64:128, 256:257].broadcast_to(
                                            [64, 64]
                                        ),
                                    )
                            for p in range(NPAIR):
                                cA = (p % 2) * 128
                                beng = nc.scalar if p % 2 == 0 else None
                                if beng is not None:
                                    beng.copy(
                                        out=bdiag[p][0:64, 0:64],
                                        in_=kvps[p][0:64, cA : cA + 64],
                                    )
                                    beng.copy(
                                        out=bdiag[p][64:128, 64:128],
                                        in_=kvps[p][64:128, cA + 64 : cA + 128],
                                    )
                                else:
                                    nc.vector.tensor_copy(
                                        out=bdiag[p][0:64, 0:64],
                                        in_=kvps[p][0:64, cA : cA + 64],
                                    )
                                    nc.vector.tensor_copy(
                                        out=bdiag[p][64:128, 64:128],
                                        in_=kvps[p][64:128, cA + 64 : cA + 128],
                                    )

            # ---------------- phase 2 (software-pipelined) ----------------
            with tc.tile_pool(name="s2", bufs=2) as s2, tc.tile_pool(
                name="ps2", bufs=1, space="PSUM"
            ) as ps2:

                def emit_recip(cn):
                    """dps matmuls for chunk cn + per-pair reciprocal straight
                    from PSUM (denominators are strictly positive and ~3e5, so
                    the reference's max(x,1e-6) clamp is a no-op)."""
                    rb = s2.tile(
                        [128, NPAIR * CH], FP32, tag="rb", name="rb", bufs=2
                    )
                    for p in range(NPAIR):
                        dps = ps2.tile([128, CH], FP32, tag="dps", bufs=2)
                        nc.tensor.matmul(
                            dps[:],
                            den_f[p][:],
                            qf[:, p * T + cn * CH : p * T + (cn + 1) * CH],
                            start=True,
                            stop=True,
                            skip_group_check=True,
                        )
                        nc.vector.reciprocal_approx_fast(
                            out=rb[:, p * CH : (p + 1) * CH], in_=dps[:]
                        )
                    return rb

                def emit_stage_e(at_l, c_):
                    tsl = slice(c_ * CH, (c_ + 1) * CH)
                    last = c_ == NCH - 1
                    obuf = s2.tile([128, 8 * CH], F16, tag="obuf", name="obuf")
                    outv = outT[:].rearrange("(g p) t -> p g t", p=128)
                    step = 1 if last else 2  # finer DMAs drain the tail faster
                    qi = 0
                    for do in range(8):
                        eps = ps2.tile([128, CH], FP32, tag="eps", bufs=3)
                        for p in range(NPAIR):
                            nc.tensor.matmul(
                                eps[:],
                                wo_t[p][:, do * 128 : (do + 1) * 128],
                                at_l[p][:],
                                start=(p == 0),
                                stop=(p == NPAIR - 1),
                            )
                        if last and do % 2 == 1:
                            nc.vector.tensor_copy(
                                out=obuf[:, do * CH : (do + 1) * CH], in_=eps[:]
                            )
                        else:
                            nc.scalar.copy(
                                out=obuf[:, do * CH : (do + 1) * CH], in_=eps[:]
                            )
                        if do % step == step - 1:
                            g0, g1 = do + 1 - step, do + 1
                            if last:
                                eng = (nc.sync, nc.gpsimd, nc.scalar)[qi % 3]
                            else:
                                eng = (nc.sync, nc.gpsimd)[qi % 2]
                            qi += 1
                            eng.dma_start(
                                out=outv[:, g0:g1, tsl],
                                in_=obuf[:, g0 * CH : g1 * CH].rearrange(
                                    "p (g t) -> p g t", g=g1 - g0
                                ),
                            )

                # warmup: reciprocal chain for chunk 0 (no numerator yet)
                rb_next = emit_recip(0)
                prev = None
                for c in range(NCH):
                    rb = rb_next
                    # numerators + scaling first: the at muls must lead the
                    # next recip chain in the engine queues so stage E is
                    # never starved.
                    at_l = []
                    for p in range(NPAIR):
                        aps = ps2.tile([128, CH], FP32, tag="aps", bufs=3)
                        nc.tensor.matmul(
                            aps[:],
                            bdiag[p][:],
                            qf[:, p * T + c * CH : p * T + (c + 1) * CH],
                            start=True,
                            stop=True,
                        )
                        at = s2.tile([128, CH], F16, tag=f"at{p}", name=f"at{p}")
                        nc.vector.tensor_mul(
                            at[:], aps[:], rb[:, p * CH : (p + 1) * CH]
                        )
                        at_l.append(at)
                    if c + 1 < NCH:
                        rb_next = emit_recip(c + 1)
                    if prev is not None:
                        emit_stage_e(*prev)
                    prev = (at_l, c)
                emit_stage_e(*prev)

    nc.finalize()
    return nc


def _warm_recip_fix(nc):
    return nc


_NC = None


def _get_nc():
    global _NC
    if _NC is None:
        _NC = _build()
    return _NC


def _rope_tables():
    """Interleaved-order rope tables.

    orig head-dim d in [0,64); interleaved position: 2j <- d=j, 2j+1 <- d=j+32.
    rope(x)[d<32] = x[d] cos - x[d+32] sin ; [d>=32] = x[d] cos + x[d-32] sin
    After interleave + XOR-1 partner:
      out[2j]   = x[2j]  * cos_j - partner * sin_j   -> sinS[2j]   = -sin_j
      out[2j+1] = x[2j+1]* cos_j + partner * sin_j   -> sinS[2j+1] = +sin_j
    """
    j = np.arange(32, dtype=np.float64)
    inv_freq = ROPE_BASE ** (-2.0 * j / HD)
    t = np.arange(T, dtype=np.float64)
    ang = t[:, None] * inv_freq[None, :]  # (T, 32)
    cos = np.cos(ang)
    sin = np.sin(ang)
    cos_i = np.empty((T, HD), np.float64)
    sinS_i = np.empty((T, HD), np.float64)
    cos_i[:, 0::2] = cos
    cos_i[:, 1::2] = cos
    sinS_i[:, 0::2] = -sin
    sinS_i[:, 1::2] = sin
    return cos_i, sinS_i


def _perm64():
    p = np.empty(HD, np.int64)
    j = np.arange(32)
    p[2 * j] = j
    p[2 * j + 1] = j + 32
    return p


def _prep_core_inputs(x, W_qkv, W_out):
    """Build the 8 per-core input maps."""
    B = x.shape[0]
    cos_i, sinS_i = _rope_tables()
    perm = _perm64()

    # (d,t)-layout q tables: stacked for the 2 heads of a pair, SCALE folded in
    cosq = np.concatenate([cos_i.T, cos_i.T], axis=0) * SCALE  # (128, T)
    sinq = np.concatenate([sinS_i.T, sinS_i.T], axis=0) * SCALE
    cosq = np.ascontiguousarray(cosq.astype(np.float16))
    sinq = np.ascontiguousarray(sinq.astype(np.float16))
    # (t,d)-layout k tables reshaped (128, 32*64): [p, s*64+d] = tab[s*128+p, d]
    cosk = np.ascontiguousarray(
        cos_i.reshape(32, 128, HD).transpose(1, 0, 2).reshape(128, 32 * HD)
    ).astype(np.float16)
    sink = np.ascontiguousarray(
        sinS_i.reshape(32, 128, HD).transpose(1, 0, 2).reshape(128, 32 * HD)
    ).astype(np.float16)

    in_maps = []
    for core in range(8):
        b, g = divmod(core, 2)
        h0 = g * H_CORE
        qcols = np.concatenate(
            [(h0 + h) * HD + perm for h in range(H_CORE)]
        )  # interleaved q columns
        kcols = 1024 + qcols
        vcols = 2048 + np.arange(h0 * HD, h0 * HD + 512)
        wq_h = np.ascontiguousarray(W_qkv[:, qcols]).astype(np.float16)
        wkv_h = np.ascontiguousarray(
            np.concatenate([W_qkv[:, kcols], W_qkv[:, vcols]], axis=1)
        ).astype(np.float16)
        wo_h = np.ascontiguousarray(W_out[h0 * HD : h0 * HD + 512, :]).astype(
            np.float16
        )
        xT_b = np.ascontiguousarray(x[b].T).astype(np.float16)
        in_maps.append(
            {
                "xT": xT_b,
                "wq": wq_h,
                "wkv": wkv_h,
                "wo": wo_h,
                "cosq": cosq,
                "sinq": sinq,
                "cosk": cosk,
                "sink": sink,
                "ones16": np.ones((128, 4), np.float16),
            }
        )
    return in_maps


def kernel(x, W_qkv, W_out):
    x = np.asarray(x, dtype=np.float32)
    W_qkv = np.asarray(W_qkv, dtype=np.float32)
    W_out = np.asarray(W_out, dtype=np.float32)
    B = x.shape[0]

    nc = _get_nc()
    in_maps = _prep_core_inputs(x, W_qkv, W_out)
    res = run_bass_kernel_spmd(nc, in_maps, core_ids=list(range(8)))

    out = np.empty((B, T, DIM), np.float32)
    for b in range(B):
        acc = res.results[2 * b]["outT"].astype(np.float32) + res.results[
            2 * b + 1
        ]["outT"].astype(np.float32)
        out[b] = acc.T
    return out
